# revision 42
# baseline (speedup 1.0000x reference)
"""MDCT (conv1d stride-512, kernel-1024, pad-512) as a Bass/Tile kernel on 8 trn2 cores.

Strategy
--------
out[b,k,j] = sum_t F[k,t] * xpad[b, j*512 + t],  x:[16,1,1048576] -> out:[16,512,2049]

* Data-parallel over batch: 2 batches per NeuronCore (8 cores).
* MDCT fold: the 2N=1024 window folds into an N=512 DCT-IV, halving matmul K:
    frame j window = [A_{j-1}, A_j]  (A_j = x[j*512:(j+1)*512])
    g2[u] = A[255-u] + A[256+u]  (u in [0,256), computed per block A)
    g1[v] = A[v]     - A[511-v]
    out[:,j] = -C'[:,0:256] @ g2(A_j) + C'[:,256:512] @ g1(A_{j-1})
  where C'[k,u] is extracted from the filter itself (least-squares over the two
  redundant copies of each coefficient present in F).
* fp16 end-to-end on device (inputs/weights/outputs cast on host): halves DMA
  bytes, halves PE transpose cost, enables DVE 2x modes. fp32 PSUM
  accumulation keeps the result well within tolerance (~5e-4 rel err).
* On-chip per (batch, 512-frame chunk): one batched x DMA [128, 4x512]
  (split per-tile for the very first chunk to shorten pipeline fill),
  fold on DVE (negative-stride reads), 16 PE identity-transposes into fp16
  PSUM (batched 4-wide per u-chunk), one DVE copy per u-chunk into MT,
  16 fp16 matmuls (4 k-chunks x 4 u-chunk accumulation) into fp32 PSUM,
  ACT cast-copies into an fp16 staging tile, one batched output DMA.
  The tail frame (j=2048, g1-only) rides in each batch's last-chunk staging
  tile; the final chunk drains with per-kc DMAs on alternating ACT/DVE.
"""

import numpy as np

N = 512
B = 16
T = 2048
NCORES = 8
BPC = B // NCORES          # batches per core = 2
JCHUNK = 512               # frames per chunk (PSUM bank = 512 fp32)
NCHUNK = T // JCHUNK       # 4 full chunks; frame 2048 rides in the last one
SAMP = N * T               # samples per batch

_compiled = None


def _build():
    import concourse.bass as bass
    import concourse.mybir as mybir
    from concourse import bacc
    from concourse.tile import TileContext
    from concourse.masks import make_identity

    f32 = mybir.dt.float32
    f16 = mybir.dt.float16

    nc = bacc.Bacc("TRN2", target_bir_lowering=False, debug=False)

    xs_d = nc.dram_tensor("xs", [BPC, SAMP], f16, kind="ExternalInput").ap()
    w_d = nc.dram_tensor("wt", [4, 128, N], f16, kind="ExternalInput").ap()
    o_d = nc.dram_tensor("os", [BPC, N, T + 1], f16, kind="ExternalOutput").ap()

    with TileContext(nc) as tc:
        with tc.tile_pool(name="wp", bufs=1) as wp, \
             tc.tile_pool(name="xp", bufs=4) as xp, \
             tc.tile_pool(name="gp", bufs=4) as gp, \
             tc.tile_pool(name="mtp", bufs=4) as mtp, \
             tc.tile_pool(name="op", bufs=4) as op, \
             tc.tile_pool(name="tps", bufs=4, space="PSUM") as tps, \
             tc.tile_pool(name="ops", bufs=4, space="PSUM") as ops:

            ident16 = wp.tile([128, 128], f16, tag="ident16")
            make_identity(nc, ident16[:])
            z0 = wp.tile([128, 1], f16, tag="z0")
            nc.vector.memset(z0[:], 0.0)
            nwarm = 24
            if nwarm:
                win = wp.tile([128, 128], f16, tag="warmin")
                nc.vector.memset(win[:], 0.0)
                # preload the ACT activation table (LoadActFuncSet ~1.3us)
                # before chunk-0's PSUM->SBUF cast-copies need it
                wact = wp.tile([128, 1], f16, tag="warmact")
                nc.scalar.copy(out=wact[:], in_=win[:, 0:1])
                wps = ops.tile([128, JCHUNK], f32, tag="po", name="wps")
                for _ in range(nwarm):
                    nc.tensor.matmul(wps[:, 0:128], win[:], win[:],
                                     start=True, stop=True)

            w_t = wp.tile([128, 4 * N], f16, tag="w")

            def W(uc, kc):
                return w_t[:, 512 * uc + 128 * kc:512 * uc + 128 * (kc + 1)]

            # startup DMA sequence: chunk(0,0) split per-tile, first half of
            # W, chunk(0,1), second half of W -- this transfer order gets
            # chunk-1's data on chip ~1.5us earlier while W still lands
            # before the matmuls that need each half (uc<2 run first).
            pending = {}
            xt00 = xp.tile([128, 4 * N], f16, tag="x", name="xt00")
            for t in range(4):
                nc.sync.dma_start(
                    out=xt00[:, 512 * t:512 * (t + 1)],
                    in_=xs_d[0, t * 128 * N:(t + 1) * 128 * N].rearrange(
                        "(p f) -> p f", p=128),
                )
            pending[(0, 0)] = xt00
            nc.sync.dma_start(
                out=w_t[:, 0:2 * N].rearrange("p (u f) -> p u f", u=2),
                in_=w_d[0:2].rearrange("u p f -> p u f"))
            xt01 = xp.tile([128, 4 * N], f16, tag="x", name="xt01")
            nc.sync.dma_start(
                out=xt01[:].rearrange("p (t f) -> p t f", t=4),
                in_=xs_d[0, JCHUNK * N:JCHUNK * N + 4 * 128 * N].rearrange(
                    "(t p f) -> p t f", t=4, p=128),
            )
            pending[(0, 1)] = xt01
            nc.sync.dma_start(
                out=w_t[:, 2 * N:4 * N].rearrange("p (u f) -> p u f", u=2),
                in_=w_d[2:4].rearrange("u p f -> p u f"))

            pieces = [(b, jc) for b in range(BPC) for jc in range(NCHUNK)]
            MTs = {}  # piece index -> MT tiles

            def prep(pi):
                """x load + fold + transposes + MT copies for piece pi."""
                b, jc = pieces[pi]
                j0 = jc * JCHUNK
                s0 = j0 * N
                x_t = pending.pop((b, jc), None)
                if x_t is None:
                    x_t = xp.tile([128, 4 * N], f16, tag="x", name="x_t")
                    nc.sync.dma_start(
                        out=x_t[:].rearrange("p (t f) -> p t f", t=4),
                        in_=xs_d[b, s0:s0 + 4 * 128 * N].rearrange(
                            "(t p f) -> p t f", t=4, p=128),
                    )
                g_t = gp.tile([128, 4 * N], f16, tag="g", name="g_t")
                for t in range(4):
                    o = 512 * t
                    nc.vector.tensor_add(
                        g_t[:, o:o + 256],
                        x_t[:, o + 255:None if o == 0 else o - 1:-1],
                        x_t[:, o + 256:o + 512])
                    nc.vector.tensor_sub(
                        g_t[:, o + 256:o + 512],
                        x_t[:, o:o + 256],
                        x_t[:, o + 511:o + 255:-1])
                MT = [mtp.tile([128, JCHUNK + 1], f16, tag=f"mt{uc}",
                               name=f"mt{uc}")
                      for uc in range(4)]
                MTs[pi] = MT
                for uc in (2, 3):
                    if jc == 0:
                        nc.vector.tensor_copy(out=MT[uc][:, 0:1], in_=z0[:])
                    else:
                        nc.vector.tensor_copy(
                            out=MT[uc][:, 0:1],
                            in_=MTs[pi - 1][uc][:, JCHUNK:JCHUNK + 1])
                for uc in range(4):
                    p_t = tps.tile([128, JCHUNK], f16, tag="tp", name="p_t")
                    for t in range(4):
                        nc.tensor.transpose(
                            p_t[:, 128 * t:128 * (t + 1)],
                            g_t[:, 512 * t + 128 * uc:512 * t + 128 * (uc + 1)],
                            ident16[:])
                    nc.vector.tensor_copy(out=MT[uc][:, 1:JCHUNK + 1],
                                          in_=p_t[:])

            def mm(pi):
                """matmuls + output staging + store for piece pi."""
                b, jc = pieces[pi]
                j0 = jc * JCHUNK
                last = jc == NCHUNK - 1
                jw = JCHUNK + 1 if last else JCHUNK
                final = pi == len(pieces) - 1
                MT = MTs[pi]
                o_t = op.tile([128, 4 * jw], f16,
                              tag="o" + ("L" if last else ""), name="o_t")
                RHS = [MT[uc][:, 1:JCHUNK + 1] if uc < 2
                       else MT[uc][:, 0:JCHUNK] for uc in range(4)]
                if pi == 0:
                    # W arrives in halves: run all uc<2 matmuls first so
                    # the strict PE FIFO never blocks on the late half.
                    # kc0 reuses the warm-up PSUM tile (start=True resets).
                    POs = [wps] + [ops.tile([128, JCHUNK], f32, tag="po",
                                   name=f"po{kc}") for kc in range(1, 4)]
                    uc_order = [(kc, uc) for uc in (0, 1) for kc in range(4)]
                    uc_order += [(kc, uc) for uc in (2, 3) for kc in range(4)]
                else:
                    POs = None
                    uc_order = None
                for kc in range(4):
                    if POs is None:
                        po = ops.tile([128, JCHUNK], f32, tag="po", name="po")
                        for uc in range(4):
                            nc.tensor.matmul(
                                po[:], W(uc, kc), RHS[uc],
                                start=(uc == 0), stop=(uc == 3),
                            )
                    else:
                        po = POs[kc]
                    cp = (nc.vector.tensor_copy if final and kc % 2
                          else nc.scalar.copy)
                    if POs is not None and kc == 0:
                        for kc2, uc in uc_order:
                            nc.tensor.matmul(
                                POs[kc2][:], W(uc, kc2), RHS[uc],
                                start=(uc == 0), stop=(uc == 3),
                            )
                    cp(out=o_t[:, jw * kc:jw * kc + JCHUNK], in_=po[:])
                    if last:
                        # tail frame j=2048: out[:,2048] = g1-side only
                        pt = ops.tile([128, JCHUNK], f32, tag="po", name="pt")
                        for i, uc in enumerate((2, 3)):
                            nc.tensor.matmul(
                                pt[:, 0:1],
                                W(uc, kc),
                                MT[uc][:, JCHUNK:JCHUNK + 1],
                                start=(i == 0), stop=(i == 1),
                            )
                        cp(out=o_t[:, jw * kc + JCHUNK:jw * (kc + 1)],
                           in_=pt[:, 0:1])
                    if final:
                        # drain chunk: per-kc DMA right after its copy
                        nc.sync.dma_start(
                            out=o_d[b, 128 * kc:128 * (kc + 1), j0:j0 + jw],
                            in_=o_t[:, jw * kc:jw * (kc + 1)],
                        )
                if not final:
                    nc.sync.dma_start(
                        out=o_d[b, :, j0:j0 + jw].rearrange(
                            "(kc p) j -> p kc j", p=128),
                        in_=o_t[:].rearrange("p (kc j) -> p kc j", kc=4),
                    )
                MTs.pop(pi - 1, None)

            # software pipeline: piece c+1's transposes sit between piece c's
            # matmuls in the PE FIFO, hiding the MT-copy semaphore latency at
            # every chunk boundary.  Piece 0/1 keep the startup-tuned order.
            prep(0)
            mm(0)
            prep(1)
            prep(2)
            for pi in range(1, len(pieces)):
                if pi + 2 < len(pieces):
                    prep(pi + 2)
                mm(pi)

    nc.compile()
    return nc


def _weights(mdct_filter: np.ndarray) -> np.ndarray:
    """Extract DCT-IV weight tiles W[4,128,512] from the 1024-tap filter.

    Each C'[k,u] coefficient appears twice in F (up to sign); average the two
    copies (least squares) to minimize the fold residual.
    """
    F = mdct_filter.reshape(N, 2 * N).astype(np.float64)
    sideA = np.concatenate([-F[:, 768:1024], F[:, 0:256]], axis=1)
    sideB = -F[:, 767:255:-1]
    Cp = 0.5 * (sideA + sideB)  # [k, u]
    W = np.empty((4, 128, 512), dtype=np.float32)
    W[0] = -Cp[:, 0:128].T
    W[1] = -Cp[:, 128:256].T
    W[2] = Cp[:, 256:384].T
    W[3] = Cp[:, 384:512].T
    return W


def kernel(x: np.ndarray, mdct_filter: np.ndarray, _trace=False) -> np.ndarray:
    global _compiled
    from concourse.bass_utils import run_bass_kernel_spmd

    if _compiled is None:
        _compiled = _build()
    nc = _compiled

    x16 = np.asarray(x, dtype=np.float32).reshape(B, SAMP).astype(np.float16)
    wt = _weights(np.asarray(mdct_filter, dtype=np.float32)).astype(np.float16)

    in_maps = [
        {"xs": x16[c * BPC:(c + 1) * BPC], "wt": wt}
        for c in range(NCORES)
    ]
    res = run_bass_kernel_spmd(nc, in_maps, core_ids=list(range(NCORES)),
                               trace=_trace)
    out = np.empty((B, N, T + 1), dtype=np.float32)
    for c in range(NCORES):
        out[c * BPC:(c + 1) * BPC] = res.results[c]["os"].astype(np.float32)
    if _trace:
        kernel._last_results = res
    return out

